# revision 1
# baseline (speedup 1.0000x reference)
"""Trainium2 Bass kernel for the CapsuleLayer dynamic-routing module.

Strategy (8 NeuronCores, data-parallel over batch, B_local = 32/core):
  - Host pre-lays-out inputs in numpy (not part of measured HW time):
      wb  [128, (i=8, jt=9, n=10, u=16)] bf16   -- W[j,n,u,i] with j = 128*jt + p
      x2  [128, (t=18, c=128)]           bf16   -- x[b,i,j]: t=(i%2)*9+jt, c=4*b+i//2
      xn  [32, 9216]                     bf16   -- x[b, (i,j)] natural
  - u_hat is never materialized. Per routing iteration:
      s-pass:  s[b,nu] = sum_{(i,j)} (W*c)[(i,j),nu] x[b,i,j] -- 72 accumulated
               PE matmuls, k=128 j-partitions, lhsT = X2 slices, rhs = A slices.
      squash on [32, 160] tiles (exact reference semantics incl. mag over n).
      a-pass:  C[(i,j),nu] = x^T v (PE, k=b=32, 9 MMs -> one 3-bank PSUM tile);
               one ACT drain per i; z = Wb .* C (DVE TT, 2x bf16); u-reduce
               via a pair-batched fold tree (two i-groups per tree, halving
               per-op overhead); incremental f32 accumulation over pairs
               (final add emits the bf16 wire tensor).
      AllGather of the [128, 90] bf16 partial agreement (exec ~8-10us vs
               AllReduce ~11-12us), wire-back as two contiguous 4-rank-block
               DMAs on separate queues, then a local 4-2-1 pairwise sum tree.
      c-pass:  cexp = exp(b); A_i = Wb_i * cexp_u (broadcast-AP dense bf16 TTs
               on DVE); D[n] via PE ones-matmul emitted after the A chain.
  - Iteration 1 uses c == uniform (A := Wb, D := 1152) and its s-pass is
    PE-bound, so the 72 matmuls are packed 4-wide into PE column groups
    (tile_position), with a block-diagonal selector matmul summing the 4
    PSUM strips.
  - An SBUF-to-SBUF remote-DMA exchange (USE_RDMA) was implemented and is
    correct for the first iteration, but measured slower end-to-end than the
    collective (it couples each core to the slowest core's full iteration-1),
    so it is disabled.
"""

import numpy as np

B, I, J, N, U = 256, 8, 1152, 10, 16
NU = N * U            # 160
ITERS = 3
NCORES = 8
BL = B // NCORES      # 32
JT = 9                # 1152 / 128
JN = JT * N           # 90

_CACHE = {}
DEBUG = False
USE_RDMA = False


def _build_nc():
    import concourse.bass as bass
    import concourse.bacc as bacc
    import concourse.tile as tile
    from concourse import mybir

    f32 = mybir.dt.float32
    bf16 = mybir.dt.bfloat16
    AL = mybir.AluOpType
    AF = mybir.ActivationFunctionType
    AX = mybir.AxisListType

    nc = bacc.Bacc("TRN2", target_bir_lowering=False, debug=False,
                   num_devices=NCORES)
    wb_d = nc.dram_tensor("wb", [128, I * JT * NU], bf16, kind="ExternalInput").ap()
    x2_d = nc.dram_tensor("x2", [128, 18 * 128], bf16, kind="ExternalInput").ap()
    xn_d = nc.dram_tensor("xn", [BL, I * J], bf16, kind="ExternalInput").ap()
    # wait thresholds for the remote-DMA agreement exchange ([14, 28]).
    # Loaded into registers at runtime: the Tile scheduling simulator reads 0
    # (no input data) so its single-core model never blocks, while hardware
    # reads the real thresholds.
    thr_d = nc.dram_tensor("thr", [1, 2], mybir.dt.int32,
                           kind="ExternalInput").ap()
    # block-diagonal selector: sel[p, b] = 1 iff p % 32 == b (reduces the 4
    # column-group strips of the iteration-1 s-pass via one PE matmul)
    sel_d = nc.dram_tensor("sel", [128, BL], bf16, kind="ExternalInput").ap()
    v_d = nc.dram_tensor("v", [BL, NU], f32, kind="ExternalOutput").ap()
    if DEBUG:
        apart_d = nc.dram_tensor("apart0", [128, JN], f32,
                                 kind="ExternalOutput").ap()
        agsum_d = nc.dram_tensor("agsum0", [128, JN], f32,
                                 kind="ExternalOutput").ap()
        vb_d = nc.dram_tensor("vb0", [BL, NU], f32,
                              kind="ExternalOutput").ap()
        cb_d = nc.dram_tensor("cb0", [128, 3 * 3 * NU], f32,
                              kind="ExternalOutput").ap()
        zt_d = nc.dram_tensor("zt0", [128, 2 * JT * N * U], f32,
                              kind="ExternalOutput").ap()
        z1_d = nc.dram_tensor("z10", [128, 2 * JT * N], f32,
                              kind="ExternalOutput").ap()

    with tile.TileContext(nc) as tc:
        with (
            tc.tile_pool(name="big", bufs=1) as big,
            tc.tile_pool(name="abp", bufs=1) as abp,
            tc.tile_pool(name="small", bufs=2) as small,
            tc.tile_pool(name="pers", bufs=1) as pers,
            tc.tile_pool(name="ps_s", bufs=1, space="PSUM") as ps_s,
            tc.tile_pool(name="ps_c", bufs=2, space="PSUM") as ps_c,
            tc.tile_pool(name="ps_d", bufs=1, space="PSUM") as ps_d,
            tc.tile_pool(name="dram", bufs=1, space="DRAM") as dram,
        ):
            # ---------------- load inputs ----------------
            X2 = big.tile([128, 18, 128], bf16)
            nc.sync.dma_start(out=X2, in_=x2_d.rearrange(
                "p (t c) -> p t c", t=18))
            wb_v = wb_d.rearrange("p (i jt n u) -> p i jt n u",
                                  i=I, jt=JT, n=N, u=U)
            Wbs = []
            for i in range(I):
                Wb_i = big.tile([128, JT, N, U], bf16, tag=f"W{i}")
                nc.sync.dma_start(out=Wb_i, in_=wb_v[:, i])
                Wbs.append(Wb_i)
            # XN only feeds the a-pass (~15us in) -- load it last
            XN = big.tile([BL, I, J], bf16)
            nc.sync.dma_start(out=XN, in_=xn_d.rearrange(
                "p (i j) -> p i j", i=I))

            ones = pers.tile([128, BL], bf16)
            nc.vector.memset(ones, 1.0)
            sel4 = pers.tile([128, BL], bf16, tag="sel4")
            nc.sync.dma_start(out=sel4, in_=sel_d)
            bmat = pers.tile([128, JN], f32)          # b[j, n] as [p, (jt, n)]
            nc.vector.memset(bmat, 0.0)

            # ---- remote-DMA agreement exchange state ----
            # Each core broadcasts its [128, 90] bf16 partial agreement
            # straight into the 7 peers' SBUF (XOR slotting: call k sends to
            # tpb my^k, landing in slot k; {m^k} covers all senders).  The
            # receive wait threshold is a register so the scheduling sim
            # (which cannot model cross-core increments) passes through.
            rsem = nc.alloc_semaphore("rsem")
            lsem = nc.alloc_semaphore("lsem")
            thr_t = pers.tile([1, 2], mybir.dt.int32, tag="thr")
            nc.sync.dma_start(out=thr_t, in_=thr_d)
            thr_regs = []
            for t in range(2):
                r = nc.vector.alloc_register(f"thr{t}")
                nc.vector.reg_load(r, thr_t[0:1, t:t + 1])
                thr_regs.append(r)
            ags0 = pers.tile([128, NCORES, JN], bf16, tag="ags0")
            ags1 = pers.tile([128, NCORES, JN], bf16, tag="ags1")
            ags_tiles = [ags0, ags1]


            # PE warm-up fodder: dependency-free matmuls the scheduler can
            # run while DMAs / collectives leave the PE idle, keeping the
            # HAM clock-gate at full rate for the real matmul bursts.
            warm_rhs = pers.tile([128, NU], bf16)
            nc.vector.memset(warm_rhs, 0.0)

            def warm_pe(count):
                # aliases the s-pass accumulator bank (tag "pss"); warm MMs
                # only run between the squash read and the next s-pass.
                pw = ps_s.tile([128, NU], f32, tag="pss")
                for w in range(count):
                    nc.tensor.matmul(pw[0:BL, :], lhsT=ones, rhs=warm_rhs,
                                     start=True, stop=True)

            warm_pe(40)

            X2v = X2.rearrange("p t (b ih) -> p t ih b", ih=4)
            XNv = XN.rearrange("b i (jt p) -> b i jt p", jt=JT)

            for it in range(ITERS):
                first = it == 0
                last = it == ITERS - 1

                # ------------ c-pass: A and Dinv ------------
                if first:
                    As = Wbs
                    Dinv_rep = small.tile([BL, N], f32, tag="dinv")
                    nc.vector.memset(Dinv_rep, 1.0 / J)
                else:
                    cexp_b = small.tile([128, JN], bf16, tag="cexpb")
                    nc.scalar.activation(out=cexp_b, in_=bmat, func=AF.Exp)
                    # cexp_u[p, jt, n, u] = cexp_b[p, jt, n]  (broadcast u)
                    cexp_u = small.tile([128, JT, N, U], bf16, tag="cexpu")
                    nc.vector.tensor_copy(
                        out=cexp_u,
                        in_=cexp_b.rearrange("p (jt n) -> p jt n", jt=JT)
                        .unsqueeze(3).broadcast_to([128, JT, N, U]))
                    cexp_flat = cexp_u.rearrange("p jt n u -> p (jt n u)")
                    As = []
                    for i in range(I):
                        A_i = abp.tile([128, JT, N, U], bf16, tag=f"A{i}")
                        nc.vector.tensor_tensor(
                            out=A_i.rearrange("p jt n u -> p (jt n u)"),
                            in0=Wbs[i].rearrange("p jt n u -> p (jt n u)"),
                            in1=cexp_flat, op=AL.mult)
                        As.append(A_i)
                    # D[n] = sum_{p,jt} cexp -> ones-matmul (replicated over
                    # the 32 batch partitions) + jt-reduce.  Emitted after the
                    # A chain so the DVE starts on A_0 first; Dinv is only
                    # consumed by the squash, well after the s-pass.
                    psd = ps_d.tile([BL, JN], f32)
                    nc.tensor.matmul(psd, lhsT=ones, rhs=cexp_b,
                                     start=True, stop=True)
                    D32 = small.tile([BL, N], f32, tag="d32")
                    nc.vector.tensor_reduce(
                        out=D32,
                        in_=psd.rearrange("q (jt n) -> q n jt", jt=JT),
                        axis=AX.X, op=AL.add)
                    Dinv_rep = small.tile([BL, N], f32, tag="dinv")
                    nc.vector.reciprocal(out=Dinv_rep, in_=D32)

                # ------------ s-pass: 72 accumulated matmuls ------------
                if first:
                    # Iteration 1 has no A-mult dependency, so the s-pass is
                    # PE-bound: pack 4 matmuls into the 128x128 array via
                    # column groups (M=32 each).  Each group accumulates 18
                    # of the 72 (i, jt) terms into its own 32-partition PSUM
                    # strip; the squash adds the 4 strips.
                    pss4 = ps_s.tile([128, NU], f32, tag="pss")
                    k = 0
                    for i in range(I):
                        il, ih = i % 2, i // 2
                        for jt in range(JT):
                            cg = k % 4
                            rnd = k // 4
                            nc.tensor.matmul(
                                pss4[32 * cg:32 * (cg + 1), :],
                                lhsT=X2v[:, il * JT + jt, ih, :],
                                rhs=As[i][:, jt, :, :],
                                start=(rnd == 0), stop=(rnd == 17),
                                tile_position=(0, 32 * cg))
                            k += 1
                else:
                    psst = ps_s.tile([128, NU], f32, tag="pss")
                    pss = psst[0:BL, :]
                    k = 0
                    for i in range(I):
                        il, ih = i % 2, i // 2
                        for jt in range(JT):
                            nc.tensor.matmul(
                                pss,
                                lhsT=X2v[:, il * JT + jt, ih, :],
                                rhs=As[i][:, jt, :, :],
                                start=(k == 0), stop=(k == 71))
                            k += 1

                # ------------ squash ------------
                if first:
                    # drain the 4 strips to SBUF, then one selector matmul
                    # sums them into a [32, 160] PSUM tile (bank reused)
                    sb4 = small.tile([128, NU], f32, tag="sb4")
                    nc.scalar.copy(out=sb4, in_=pss4)
                    sb4b = small.tile([128, NU], bf16, tag="sb4b")
                    nc.vector.tensor_copy(out=sb4b, in_=sb4)
                    pfin = ps_s.tile([128, NU], f32, tag="pss")
                    nc.tensor.matmul(pfin[0:BL, :], lhsT=sel4, rhs=sb4b,
                                     start=True, stop=True)
                    pss = pfin[0:BL, :]
                s_sc = small.tile([BL, N, U], f32, tag="ssc")
                nc.vector.tensor_tensor(
                    out=s_sc,
                    in0=pss.rearrange("b (n u) -> b n u", n=N),
                    in1=Dinv_rep.unsqueeze(2).broadcast_to([BL, N, U]),
                    op=AL.mult)
                sq = small.tile([BL, N, U], f32, tag="sq")
                nc.vector.tensor_tensor(out=sq, in0=s_sc, in1=s_sc,
                                        op=AL.mult)
                mag = small.tile([BL, U], f32, tag="mag")
                nc.vector.tensor_reduce(
                    out=mag, in_=sq.rearrange("b n u -> b u n"),
                    axis=AX.X, op=AL.add)
                sqrtm = small.tile([BL, U], f32, tag="sqrtm")
                nc.scalar.activation(out=sqrtm, in_=mag, func=AF.Sqrt)
                onep = small.tile([BL, U], f32, tag="onep")
                nc.vector.tensor_scalar_add(out=onep, in0=mag, scalar1=1.0)
                rec = small.tile([BL, U], f32, tag="rec")
                nc.vector.reciprocal(out=rec, in_=onep)
                g = small.tile([BL, U], f32, tag="g")
                if last:
                    nc.vector.tensor_tensor(out=g, in0=sqrtm, in1=rec,
                                            op=AL.mult)
                    v_f32 = small.tile([BL, N, U], f32, tag="vf32")
                    nc.vector.tensor_tensor(
                        out=v_f32, in0=s_sc,
                        in1=g.unsqueeze(1).broadcast_to([BL, N, U]),
                        op=AL.mult)
                    nc.sync.dma_start(
                        out=v_d, in_=v_f32.rearrange("b n u -> b (n u)"))
                    break

                # fold the 1/B mean scale into g; emit bf16 v directly
                nc.vector.scalar_tensor_tensor(
                    out=g, in0=sqrtm, scalar=1.0 / B, in1=rec,
                    op0=AL.mult, op1=AL.mult)
                vb16 = small.tile([BL, N, U], bf16, tag="vb16")
                nc.vector.tensor_tensor(
                    out=vb16, in0=s_sc,
                    in1=g.unsqueeze(1).broadcast_to([BL, N, U]),
                    op=AL.mult)
                vb16 = vb16.rearrange("b n u -> b (n u)")

                # ------------ a-pass ------------
                # Per i: 9 C-matmuls into one 3-bank PSUM tile -> one ACT
                # drain -> DVE product (2x bf16).  The u-fold runs on PAIRS
                # of i (halves the per-op fixed overhead vs per-i trees);
                # incremental f32 accumulate over pairs, last emits the bf16
                # wire tensor.
                aacc = small.tile([128, JN], f32, tag="aacc")
                apart = small.tile([128, JN], bf16, tag="apart")
                with nc.allow_low_precision(
                        reason="agreement wire format; 8-term add held in "
                               "f32, bf16 rounding is within tolerance"):
                    for i in range(I):
                        psc = ps_c.tile([128, 3, 512], f32)
                        for jt in range(JT):
                            gb, kb = divmod(jt, 3)
                            nc.tensor.matmul(
                                psc[:, gb, kb * NU:(kb + 1) * NU],
                                lhsT=XNv[:, i, jt, :],
                                rhs=vb16,
                                start=True, stop=True)
                        Cb_i = small.tile([128, 3, 3 * NU], bf16,
                                          tag=f"cb{i % 2}")
                        nc.scalar.copy(out=Cb_i, in_=psc[:, :, 0:3 * NU])
                        if DEBUG and it == 0 and i == 0:
                            cbf = small.tile([128, 3, 3 * NU], f32,
                                             tag="dbg_cb")
                            nc.vector.tensor_copy(out=cbf, in_=Cb_i)
                            nc.sync.dma_start(
                                out=cb_d,
                                in_=cbf.rearrange("p a b -> p (a b)"))
                        if i % 2 == 0:
                            zt = small.tile([128, 2 * JT, N, U], bf16,
                                            tag="zpair")
                        nc.vector.tensor_tensor(
                            out=zt[:, (i % 2) * JT:(i % 2 + 1) * JT]
                            .rearrange("p jt n u -> p (jt n u)"),
                            in0=Wbs[i].rearrange("p jt n u -> p (jt n u)"),
                            in1=Cb_i.rearrange("p g r -> p (g r)"),
                            op=AL.mult)
                        if i % 2 == 0:
                            continue
                        t8 = small.tile([128, 2 * JT, N, 8], bf16, tag="t8")
                        nc.vector.tensor_tensor(
                            out=t8, in0=zt[:, :, :, 0:8],
                            in1=zt[:, :, :, 8:16], op=AL.add)
                        t4 = small.tile([128, 2 * JT, N, 4], bf16, tag="t4")
                        nc.vector.tensor_tensor(
                            out=t4, in0=t8[:, :, :, 0:4],
                            in1=t8[:, :, :, 4:8], op=AL.add)
                        t2 = small.tile([128, 2 * JT, N, 2], bf16, tag="t2")
                        nc.vector.tensor_tensor(
                            out=t2, in0=t4[:, :, :, 0:2],
                            in1=t4[:, :, :, 2:4], op=AL.add)
                        z1 = small.tile([128, 2 * JT, N], bf16, tag="z1")
                        nc.vector.tensor_tensor(
                            out=z1, in0=t2[:, :, :, 0],
                            in1=t2[:, :, :, 1], op=AL.add)
                        ps = small.tile([128, JN], bf16, tag="psum2")
                        nc.vector.tensor_tensor(
                            out=ps,
                            in0=z1[:, 0:JT].rearrange("p a b -> p (a b)"),
                            in1=z1[:, JT:2 * JT].rearrange("p a b -> p (a b)"),
                            op=AL.add)
                        if DEBUG and it == 0 and i == 1:
                            ztf = small.tile([128, 2 * JT * N * U], f32,
                                             tag="dbg_zt")
                            nc.vector.tensor_copy(
                                out=ztf,
                                in_=zt.rearrange("p a b c -> p (a b c)"))
                            nc.sync.dma_start(out=zt_d, in_=ztf)
                            z1f = small.tile([128, 2 * JT * N], f32,
                                             tag="dbg_z1")
                            nc.vector.tensor_copy(
                                out=z1f,
                                in_=z1.rearrange("p a b -> p (a b)"))
                            nc.sync.dma_start(out=z1_d, in_=z1f)
                        if i == 1:
                            nc.vector.tensor_copy(out=aacc, in_=ps)
                        elif i < I - 1:
                            nc.vector.tensor_tensor(out=aacc, in0=aacc,
                                                    in1=ps, op=AL.add)
                        else:
                            nc.vector.tensor_tensor(out=apart, in0=aacc,
                                                    in1=ps, op=AL.add)

                    if it == 0 and USE_RDMA:
                        # ---- iteration 1: remote-DMA exchange ----
                        # Each core broadcasts its partial straight into the
                        # 7 peers' SBUF (no ncfw collective, no startup
                        # barrier on the critical path).  A second exchange
                        # round on the same SWDGE ring misbehaves, so
                        # iteration 2 uses the collective path instead.
                        ags_t = ags_tiles[0]
                        for k in range(1, NCORES):
                            rd = [None] * NCORES
                            rd[k] = (0, k)
                            nc.gpsimd.remote_dma_broadcast(
                                out_ap=ags_t[:, k, :], in_ap=apart,
                                remote_sem=rsem, local_sem=lsem, rdests=rd)
                        nc.gpsimd.trigger_dma(count=None)
                        # Sequential accumulation STARTING from the local
                        # apart: every op transitively depends on the local
                        # a-pass, so the scheduler cannot hoist the rsem wait
                        # ahead of the folds (which would deadlock the ring).
                        prev = apart
                        for k in range(1, NCORES):
                            acc = small.tile([128, JN], f32, tag=f"agacc{k}")
                            inst = nc.vector.tensor_tensor(
                                out=acc, in0=prev, in1=ags_t[:, k, :],
                                op=AL.add)
                            if k == 1:
                                inst.wait_op(rsem, thr_regs[0], "sem-ge")
                            prev = acc
                    else:
                        # ---- iteration 2: ncfw AllGather collective ----
                        ag_in = dram.tile([128, JN], bf16, tag=f"agi{it}")
                        ag_out = dram.tile([NCORES * 128, JN], bf16,
                                           tag=f"ago{it}")
                        nc.sync.dma_start(out=ag_in, in_=apart)
                        nc.gpsimd.collective_compute(
                            "AllGather", AL.bypass,
                            ins=[ag_in.opt()], outs=[ag_out.opt()],
                            replica_groups=[list(range(NCORES))])
                        # contiguous 2-rank-block wire-back DMAs spread over
                        # four engine queues (the single strided gather costs
                        # ~3.4us in 180B packets)
                        ags_t = ags_tiles[it]
                        agv = ag_out.rearrange("(r p) f -> r p f", p=128)
                        splits = [(0, 3, nc.sync), (3, 6, nc.scalar),
                                  (6, 8, nc.gpsimd)]
                        for lo, hi, eng in splits:
                            eng.dma_start(
                                out=ags_t[:, lo:hi, :],
                                in_=agv[lo:hi].rearrange("r p f -> p r f"))
                        t1 = small.tile([128, 4, JN], bf16, tag="agt1")
                        nc.vector.tensor_tensor(out=t1, in0=ags_t[:, 0:4],
                                                in1=ags_t[:, 4:8], op=AL.add)
                        t2 = small.tile([128, 2, JN], bf16, tag="agt2")
                        nc.vector.tensor_tensor(out=t2, in0=t1[:, 0:2],
                                                in1=t1[:, 2:4], op=AL.add)
                        t3 = small.tile([128, JN], bf16, tag="agt3")
                        nc.vector.tensor_tensor(out=t3, in0=t2[:, 0],
                                                in1=t2[:, 1], op=AL.add)
                        prev = t3
                nc.vector.tensor_tensor(out=bmat, in0=bmat, in1=prev,
                                        op=AL.add)
                if DEBUG and it == 0:
                    apf = small.tile([128, JN], f32, tag="dbg_ap")
                    nc.vector.tensor_copy(out=apf, in_=apart)
                    nc.sync.dma_start(out=apart_d, in_=apf)
                    agf = small.tile([128, JN], f32, tag="dbg_ag")
                    nc.vector.tensor_copy(out=agf, in_=prev)
                    nc.sync.dma_start(out=agsum_d, in_=agf)
                    vbf = small.tile([BL, NU], f32, tag="dbg_vb")
                    nc.vector.tensor_copy(out=vbf, in_=vb16)
                    nc.sync.dma_start(out=vb_d, in_=vbf)
                warm_pe(30)     # keep the PE warm through the AllGather wait

    nc.compile()
    return nc


def _prep_inputs(x_full, W):
    """Host-side relayout. x_full: [B, I, J] f32, W: [J, N, U, I] f32."""
    import ml_dtypes
    bf = ml_dtypes.bfloat16
    # Wb[p, i, jt, n, u] = W[128*jt+p, n, u, i]
    Wb = np.ascontiguousarray(
        W.reshape(JT, 128, N, U, I).transpose(1, 4, 0, 2, 3)
    ).reshape(128, I * JT * N * U).astype(bf)
    in_maps = []
    for c in range(NCORES):
        xc = x_full[c * BL:(c + 1) * BL]                   # [32, 8, 1152]
        # x128[4b+ih, il, j] = xc[b, 2*ih+il, j]
        x128 = xc.reshape(BL, 4, 2, J).reshape(128, 2, J)
        # X2[p, t=(il*9+jt), c] = x128[c, il, 128*jt+p]
        X2 = np.ascontiguousarray(
            x128.reshape(128, 2, JT, 128).transpose(3, 1, 2, 0)
        ).reshape(128, 18 * 128).astype(bf)
        xn = xc.reshape(BL, I * J).astype(bf)
        thr = np.array([[14, 28]], dtype=np.int32)
        sel = np.tile(np.eye(BL, dtype=np.float32), (4, 1)).astype(bf)
        in_maps.append({"wb": Wb, "x2": X2, "xn": xn, "thr": thr,
                        "sel": sel})
    return in_maps


def kernel(x, W):
    """x: [256, 8, 1152] f32; W: [1152, 10, 16, 8] f32 ->
    v: [256, 10, 16, 1] f32."""
    from concourse.bass_utils import run_bass_kernel_spmd

    x = np.asarray(x, dtype=np.float32)
    W = np.asarray(W, dtype=np.float32)
    if "nc" not in _CACHE:
        _CACHE["nc"] = _build_nc()
    nc = _CACHE["nc"]
    in_maps = _prep_inputs(x, W)
    res = run_bass_kernel_spmd(nc, in_maps, core_ids=list(range(NCORES)))
    out = np.concatenate([r["v"] for r in res.results], axis=0)
    return out.reshape(B, N, U, 1).astype(np.float32)


if __name__ == "__main__":
    rng = np.random.default_rng(0)
    x = rng.standard_normal((B, I, J), dtype=np.float32)
    W = rng.standard_normal((J, N, U, I), dtype=np.float32)
    got = kernel(x, W)
    # numpy reference for a self-contained smoke test
    u_hat = np.einsum('jnui,bij->bjnu', W, x)
    b = np.zeros((J, N), dtype=np.float32)
    for _ in range(ITERS):
        e = np.exp(b - b.max(axis=0, keepdims=True))
        c = e / e.sum(axis=0, keepdims=True)
        s = np.einsum('jn,bjnu->bnu', c, u_hat)
        mag = np.sum(s * s, axis=1, keepdims=True)
        v = (mag / (1.0 + mag)) * (s / np.sqrt(mag))
        b = b + np.einsum('bjnu,bnu->jn', u_hat, v) / B
    exp = v[..., None]
    rel = np.linalg.norm(got - exp) / np.linalg.norm(exp)
    print("rel_fro:", rel)

